# revision 1
# baseline (speedup 1.0000x reference)
"""Deformable-ROI bilinear feature gather (KeypPointBboxNet) on 8 TRN2 cores.

Strategy (matches the sharding hint): feat_map is sharded on the batch dim
(one image per NeuronCore, in HWC layout so a pixel's C=256 channels are
contiguous); rois/offsets are routed host-side to the core holding their
image. On-device per core:
  - compute bilinear sample coordinates + weights from the routed roi/offset
    fields (DVE), in the two layouts the hardware needs them in,
  - dma_gather (SWDGE) pulls, per sample point, the two 2KB pixel-pairs
    [(hl,wl),(hl,wl+1)] and [(hl+1,wl),(hl+1,wl+1)] straight out of the
    HBM-resident image,
  - ACT+DVE combine them into the bilinear result,
  - one linear DMA stores the result; the host inverse-routes to full shape.
"""

import math

import numpy as np

B, C, H, W = 8, 256, 128, 128
N_ROIS, NUM_POINT, STRIDE = 2048, 9, 8
NCORES = 8
SG = 5  # slots (of 128 points) per dma_gather call
# fm rows addressable by gathers: idx_bot can reach H*W + W - 1 = 16511 and
# each gather reads 2 rows -> pad the image to 16640 rows of zeros.
FM_ROWS = H * W + 2 * W
FM_VIEW_ROWS = FM_ROWS - 1  # max start row such that a 2-row read stays in bounds

_PROGRAM_CACHE: dict[int, object] = {}


def _build_program(S: int, iters: int = 1):
    import concourse.bacc as bacc
    import concourse.mybir as mybir
    import concourse.tile as tile
    from concourse.bass_types import AP

    f32 = mybir.dt.float32
    i32 = mybir.dt.int32
    i16 = mybir.dt.int16
    op = mybir.AluOpType
    G = S // SG

    nc = bacc.Bacc("TRN2", target_bir_lowering=False, debug=False, num_devices=NCORES)
    fm_t = nc.dram_tensor("fm", [FM_ROWS, C], f32, kind="ExternalInput")
    pt16_t = nc.dram_tensor("pt16", [16, 8 * S * 6], f32, kind="ExternalInput")
    pt128_t = nc.dram_tensor("pt128", [128, S * 6], f32, kind="ExternalInput")
    out_t = nc.dram_tensor("out", [128, S * C], f32, kind="ExternalOutput")

    # fm viewed as overlapping [row, 2*C] rows with stride C (so one gathered
    # element covers pixels (h,w) and (h,w+1)).
    fm_gather_ap = AP(fm_t, 0, [[C, FM_VIEW_ROWS], [1, 2 * C]])

    with tile.TileContext(nc) as tc:
        with (
            tc.tile_pool(name="const", bufs=1) as cpool,
            tc.tile_pool(name="gath", bufs=2) as gpool,
            tc.tile_pool(name="work", bufs=3) as wpool,
        ):
            p16 = cpool.tile([16, 8 * S * 6], f32)
            nc.sync.dma_start(p16[:], pt16_t[:])
            p128 = cpool.tile([128, S * 6], f32)
            nc.sync.dma_start(p128[:], pt128_t[:])

            v16 = p16[:].rearrange("p (q f) -> p q f", f=6)
            v128 = p128[:].rearrange("p (q f) -> p q f", f=6)

            def coord_chain(v, P, Q, axis, want_weight):
                """Per-point sample coordinate along one axis.

                v: [P, Q, 6] point fields (x1,y1,x2,y2,ox,oy). Returns
                (ccf, lw): ccf = clip(floor(coord),0,127)+16 as f32,
                lw = fractional weight (edge rules applied) or None.
                """
                lo = v[:, :, 0 + axis]
                hi = v[:, :, 2 + axis]
                off = v[:, :, 4 + axis]
                w0 = wpool.tile([P, Q], f32, tag=f"w0{axis}{P}")
                nc.vector.scalar_tensor_tensor(w0[:], lo, -1.0, hi, op.mult, op.add)
                sx = wpool.tile([P, Q], f32, tag=f"sx{axis}{P}")
                nc.vector.tensor_scalar(sx[:], w0[:], 1.0, 0.1 / STRIDE, op.add, op.mult)
                asum = wpool.tile([P, Q], f32, tag=f"as{axis}{P}")
                nc.vector.tensor_tensor(asum[:], lo, hi, op.add)
                ax = wpool.tile([P, Q], f32, tag=f"ax{axis}{P}")
                nc.vector.tensor_scalar(ax[:], asum[:], 0.5 / STRIDE, 16.0, op.mult, op.add)
                ixs = wpool.tile([P, Q], f32, tag=f"ix{axis}{P}")
                nc.vector.tensor_tensor(ixs[:], off, sx[:], op.mult)
                nc.vector.tensor_tensor(ixs[:], ixs[:], ax[:], op.add)  # coord+16
                ci = wpool.tile([P, Q], i32, tag=f"ci{axis}{P}")
                nc.vector.tensor_copy(ci[:], ixs[:])
                cif = wpool.tile([P, Q], f32, tag=f"cf{axis}{P}")
                nc.vector.tensor_copy(cif[:], ci[:])
                gt = wpool.tile([P, Q], f32, tag=f"gt{axis}{P}")
                nc.vector.tensor_tensor(gt[:], cif[:], ixs[:], op.is_gt)
                nc.vector.tensor_tensor(cif[:], cif[:], gt[:], op.subtract)  # floor+16
                nc.vector.tensor_scalar(cif[:], cif[:], 143.0, 16.0, op.min, op.max)
                if not want_weight:
                    return cif, None
                d = wpool.tile([P, Q], f32, tag=f"d{axis}{P}")
                nc.vector.tensor_tensor(d[:], ixs[:], cif[:], op.subtract)
                m = wpool.tile([P, Q], f32, tag=f"m{axis}{P}")
                nc.vector.tensor_scalar(m[:], cif[:], 143.0, None, op.is_lt)
                nc.vector.tensor_tensor(d[:], d[:], m[:], op.mult)
                return cif, d

            # --- index pipeline in [16, 8S] layout (the dma_gather idx layout)
            ccx16, _ = coord_chain(v16, 16, 8 * S, 0, False)
            ccy16, _ = coord_chain(v16, 16, 8 * S, 1, False)
            idxf = wpool.tile([16, 8 * S], f32, tag="idxf")
            nc.vector.scalar_tensor_tensor(idxf[:], ccy16[:], float(W), ccx16[:], op.mult, op.add)
            idxt_f = wpool.tile([16, 8 * S], f32, tag="idxtf")
            nc.vector.tensor_scalar(idxt_f[:], idxf[:], -(16.0 * W + 16.0), None, op.add)
            idxb_f = wpool.tile([16, 8 * S], f32, tag="idxbf")
            nc.vector.tensor_scalar(idxb_f[:], idxf[:], -(16.0 * W + 16.0) + W, None, op.add)

            idx = cpool.tile([128, 2, 8 * S], i16)
            nc.vector.tensor_copy(idx[0:16, 0, :], idxt_f[:])
            nc.vector.tensor_copy(idx[0:16, 1, :], idxb_f[:])
            # replicate the [16, *] index band to all 128 partitions (each of
            # the 8 gpsimd cores reads its own 16-partition stripe)
            nc.sync.dma_start(idx[16:32], idx[0:16])
            nc.sync.dma_start(idx[32:64], idx[0:32])
            nc.sync.dma_start(idx[64:128], idx[0:64])

            # --- weight pipeline in [128, S] layout (per-partition scalars)
            _, lw = coord_chain(v128, 128, S, 0, True)
            _, lh = coord_chain(v128, 128, S, 1, True)
            ch = cpool.tile([128, S], f32)
            nc.vector.tensor_scalar(ch[:], lh[:], -1.0, 1.0, op.mult, op.add)

            outt = cpool.tile([128, S * C], f32)

            for _it in range(iters):
              for g in range(G):
                    tt = gpool.tile([128, SG, 2 * C], f32, tag="tt")
                    bt = gpool.tile([128, SG, 2 * C], f32, tag="bt")
                    ncols = 8 * SG
                    nc.gpsimd.dma_gather(
                        tt[:], fm_gather_ap, idx[:, 0, g * ncols : (g + 1) * ncols],
                        SG * 128, SG * 128, 2 * C, elem_step=C,
                    )
                    nc.gpsimd.dma_gather(
                        bt[:], fm_gather_ap, idx[:, 1, g * ncols : (g + 1) * ncols],
                        SG * 128, SG * 128, 2 * C, elem_step=C,
                    )
                    for sl in range(SG):
                        s = g * SG + sl
                        t1 = wpool.tile([128, 2 * C], f32, tag="t1")
                        nc.scalar.activation(
                            t1[:], tt[:, sl, :], mybir.ActivationFunctionType.Copy,
                            bias=0.0, scale=ch[:, s : s + 1],
                        )
                        st = wpool.tile([128, 2 * C], f32, tag="st")
                        nc.vector.scalar_tensor_tensor(
                            st[:], bt[:, sl, :], lh[:, s : s + 1], t1[:], op.mult, op.add
                        )
                        d = wpool.tile([128, C], f32, tag="dd")
                        nc.vector.tensor_tensor(d[:], st[:, C : 2 * C], st[:, 0:C], op.subtract)
                        nc.vector.scalar_tensor_tensor(
                            outt[:, s * C : (s + 1) * C], d[:], lw[:, s : s + 1],
                            st[:, 0:C], op.mult, op.add,
                        )

            nc.sync.dma_start(out_t[:], outt[:])

    nc.compile()
    return nc


def _get_program(S: int):
    if S not in _PROGRAM_CACHE:
        _PROGRAM_CACHE[S] = _build_program(S)
    return _PROGRAM_CACHE[S]


def _host_prep(feat_map, rois, offset, num_point):
    """Route rois by batch index; build per-core inputs."""
    bidx = rois[:, 0].astype(np.int32)
    ids = [np.nonzero(bidx == b)[0] for b in range(B)]
    cap = max(len(i) for i in ids)
    S = math.ceil(max(cap * num_point, 1) / 128)
    S = ((S + SG - 1) // SG) * SG
    NP = S * 128

    in_maps = []
    for b in range(B):
        fmb = np.ascontiguousarray(feat_map[b].transpose(1, 2, 0)).reshape(H * W, C)
        fm_full = np.zeros((FM_ROWS, C), np.float32)
        fm_full[: H * W] = fmb
        ptdata = np.zeros((NP, 6), np.float32)
        idl = ids[b]
        nb = len(idl)
        if nb:
            r = rois[idl]
            off = offset[idl].reshape(nb, num_point, 2)
            npts = nb * num_point
            ptdata[:npts, 0] = np.repeat(r[:, 1], num_point)
            ptdata[:npts, 1] = np.repeat(r[:, 2], num_point)
            ptdata[:npts, 2] = np.repeat(r[:, 3], num_point)
            ptdata[:npts, 3] = np.repeat(r[:, 4], num_point)
            ptdata[:npts, 4] = off[:, :, 0].reshape(-1)
            ptdata[:npts, 5] = off[:, :, 1].reshape(-1)
        pt128 = np.ascontiguousarray(
            ptdata.reshape(S, 128, 6).transpose(1, 0, 2)
        ).reshape(128, S * 6)
        pt16 = np.ascontiguousarray(
            ptdata.reshape(S * 8, 16, 6).transpose(1, 0, 2)
        ).reshape(16, S * 8 * 6)
        in_maps.append({"fm": fm_full, "pt16": pt16, "pt128": pt128})
    return ids, S, in_maps


def _host_unshard(results, ids, S, num_point, n):
    out_full = np.zeros((n, num_point, C), np.float32)
    for b in range(B):
        nb = len(ids[b])
        if not nb:
            continue
        o = results[b]["out"].reshape(128, S, C).transpose(1, 0, 2).reshape(S * 128, C)
        out_full[ids[b]] = o[: nb * num_point].reshape(nb, num_point, C)
    return out_full


def kernel(feat_map, rois, offset, stride, num_point, _collect=None):
    from concourse.bass_utils import run_bass_kernel_spmd

    feat_map = np.ascontiguousarray(np.asarray(feat_map, np.float32))
    rois = np.asarray(rois, np.float32)
    offset = np.asarray(offset, np.float32)
    stride = int(stride)
    num_point = int(num_point)
    assert feat_map.shape == (B, C, H, W), feat_map.shape
    assert stride == STRIDE and num_point == NUM_POINT

    ids, S, in_maps = _host_prep(feat_map, rois, offset, num_point)
    nc = _get_program(S)
    res = run_bass_kernel_spmd(nc, in_maps, core_ids=list(range(NCORES)),
                               **(_collect.pop("spmd_kwargs", {}) if _collect else {}))
    if _collect is not None:
        _collect["res"] = res
    return _host_unshard(res.results, ids, S, num_point, rois.shape[0])



# revision 2
# speedup vs baseline: 3.9848x; 3.9848x over previous
"""Deformable-ROI bilinear feature gather (KeypPointBboxNet) on 8 TRN2 cores.

Strategy: fp16 feature map in HWC layout, two images packed per core (each
image replicated on 2 cores) so an int16 pixel index (max 32767 = 2*16384-1)
addresses the whole pair; the (roi,point) list of each image pair is split
point-wise between its two cores for near-perfect load balance. The host
precomputes, per point, the int16 gather index (h_low*W + w_low + img_off)
and the four fp16 bilinear corner weights, laid out exactly as the device
needs them. On device, per 640-point group:
  - one dma_gather (SWDGE) pulls the top pixel-pair (h, w..w+1) = 1KB,
  - a second dma_gather with the same indices but an AP base offset of +W
    pixels pulls the bottom pixel-pair (no separate bottom index needed),
  - ACT + three fused scalar_tensor_tensor ops (all fp16, single-SBUF-port
    2x mode, so SWDGE descriptor generation is never port-starved) combine
    the four corners: out = w1*v1 + w2*v2 + w3*v3 + w4*v4.
One linear DMA stores the fp16 result; the host inverse-routes to f32 full
shape.
"""

import math

import numpy as np

B, C, H, W = 8, 256, 128, 128
N_ROIS, NUM_POINT, STRIDE = 2048, 9, 8
NCORES = 8
SG = 5  # slots (of 128 points) per dma_gather call
IMG_PIX = H * W
# two images per core + 129 pad rows (bottom gather at the last row of image 1
# reads pixels up to 2*IMG_PIX + 128 + 1).
PIX_ROWS = 2 * IMG_PIX + 2 * W + 1

_PROGRAM_CACHE: dict[int, object] = {}


def _build_program(S: int, iters: int = 1):
    import concourse.bacc as bacc
    import concourse.mybir as mybir
    import concourse.tile as tile
    from concourse.bass_types import AP

    f32 = mybir.dt.float32
    f16 = mybir.dt.float16
    i16 = mybir.dt.int16
    op = mybir.AluOpType
    Act = mybir.ActivationFunctionType

    nc = bacc.Bacc("TRN2", target_bir_lowering=False, debug=False, num_devices=NCORES)
    fm_t = nc.dram_tensor("fm", [PIX_ROWS, C], f16, kind="ExternalInput")
    idx_t = nc.dram_tensor("idx", [128, 8 * S], i16, kind="ExternalInput")
    w1_t = nc.dram_tensor("w1", [128, S], f32, kind="ExternalInput")
    w_t = nc.dram_tensor("w", [128, 3 * S], f16, kind="ExternalInput")
    out_t = nc.dram_tensor("out", [128, S * C], f16, kind="ExternalOutput")

    # fm viewed as overlapping [pixel, 2*C] rows with stride C (one gathered
    # element covers pixels (h,w) and (h,w+1)); the bottom view starts W
    # pixels later so the same index reads row h+1.
    top_ap = AP(fm_t, 0, [[C, 2 * IMG_PIX], [1, 2 * C]])
    bot_ap = AP(fm_t, W * C, [[C, 2 * IMG_PIX], [1, 2 * C]])

    groups = []
    s0 = 0
    while s0 < S:
        groups.append((s0, min(SG, S - s0)))
        s0 += SG

    with tile.TileContext(nc) as tc:
        with (
            tc.tile_pool(name="const", bufs=1) as cpool,
            tc.tile_pool(name="gath", bufs=2) as gpool,
            tc.tile_pool(name="work", bufs=4) as wpool,
        ):
            idx = cpool.tile([128, 8 * S], i16)
            nc.sync.dma_start(idx[:], idx_t[:])
            w1 = cpool.tile([128, S], f32)
            nc.sync.dma_start(w1[:], w1_t[:])
            wt = cpool.tile([128, 3 * S], f16)
            nc.sync.dma_start(wt[:], w_t[:])
            outt = cpool.tile([128, S * C], f16)

            for _it in range(iters):
                for g0, gs in groups:
                    tt = gpool.tile([128, SG, 2 * C], f16, tag="tt")
                    bt = gpool.tile([128, SG, 2 * C], f16, tag="bt")
                    nidx = gs * 128
                    isl = idx[:, 8 * g0 : 8 * g0 + 8 * gs]
                    nc.gpsimd.dma_gather(
                        tt[:, 0:gs, :], top_ap, isl, nidx, nidx, 2 * C, elem_step=C
                    )
                    nc.gpsimd.dma_gather(
                        bt[:, 0:gs, :], bot_ap, isl, nidx, nidx, 2 * C, elem_step=C
                    )
                    for sl in range(gs):
                        s = g0 + sl
                        t = wpool.tile([128, C], f16, tag="t0")
                        nc.scalar.activation(
                            t[:], tt[:, sl, 0:C], Act.Copy,
                            bias=0.0, scale=w1[:, s : s + 1],
                        )
                        a = wpool.tile([128, C], f16, tag="a0")
                        nc.vector.scalar_tensor_tensor(
                            a[:], tt[:, sl, C : 2 * C], wt[:, s : s + 1],
                            t[:], op.mult, op.add,
                        )
                        b = wpool.tile([128, C], f16, tag="b0")
                        nc.vector.scalar_tensor_tensor(
                            b[:], bt[:, sl, 0:C], wt[:, S + s : S + s + 1],
                            a[:], op.mult, op.add,
                        )
                        nc.vector.scalar_tensor_tensor(
                            outt[:, s * C : (s + 1) * C], bt[:, sl, C : 2 * C],
                            wt[:, 2 * S + s : 2 * S + s + 1], b[:], op.mult, op.add,
                        )

            nc.sync.dma_start(out_t[:], outt[:])

    nc.compile()
    return nc


def _get_program(S: int):
    if S not in _PROGRAM_CACHE:
        _PROGRAM_CACHE[S] = _build_program(S)
    return _PROGRAM_CACHE[S]


def _host_prep(feat_map, rois, offset, num_point):
    """Pair images, split each pair's points across its 2 cores, and build
    per-core fm / idx / weight tensors in the exact device layouts."""
    n = rois.shape[0]
    bidx = rois[:, 0].astype(np.int32)
    cnt = np.bincount(bidx, minlength=B)
    order = np.argsort(-cnt, kind="stable")
    pairs = [(int(order[k]), int(order[B - 1 - k])) for k in range(B // 2)]

    # per-point bilinear coords/weights for ALL rois (f32, matches reference)
    x1 = rois[:, 1]
    y1 = rois[:, 2]
    x2 = rois[:, 3]
    y2 = rois[:, 4]
    cx = (x1 + x2) / 2
    cy = (y1 + y2) / 2
    wx = x2 - x1 + 1
    wy = y2 - y1 + 1
    off = offset.reshape(n, num_point, 2)
    ix = (cx[:, None] + off[:, :, 0] * wx[:, None] * np.float32(0.1)) / np.float32(
        STRIDE
    )
    iy = (cy[:, None] + off[:, :, 1] * wy[:, None] * np.float32(0.1)) / np.float32(
        STRIDE
    )
    wl = np.clip(np.floor(ix), 0.0, W - 1).astype(np.float32)
    hl = np.clip(np.floor(iy), 0.0, H - 1).astype(np.float32)
    lw = np.where(wl >= W - 1, np.float32(0.0), (ix - wl).astype(np.float32))
    lh = np.where(hl >= H - 1, np.float32(0.0), (iy - hl).astype(np.float32))
    w1 = (1 - lh) * (1 - lw)
    w2 = (1 - lh) * lw
    w3 = lh * (1 - lw)
    w4 = lh * lw
    pix = (hl * W + wl).astype(np.int32)  # [n, P] in [0, IMG_PIX)

    # point-level assignment: pair k -> cores 2k, 2k+1
    sel_r, sel_j, sel_pix = [], [], []
    for a, b in pairs:
        ra = np.nonzero(bidx == a)[0]
        rb = np.nonzero(bidx == b)[0]
        rr = np.concatenate([ra, rb])
        im = np.concatenate(
            [np.zeros(len(ra), np.int32), np.ones(len(rb), np.int32)]
        )
        roi_rep = np.repeat(rr, num_point)
        img_rep = np.repeat(im, num_point)
        pt_j = np.tile(np.arange(num_point), len(rr))
        p = pix[roi_rep, pt_j] + img_rep * IMG_PIX
        half = (len(roi_rep) + 1) // 2
        for lo, hi in ((0, half), (half, len(roi_rep))):
            sel_r.append(roi_rep[lo:hi])
            sel_j.append(pt_j[lo:hi])
            sel_pix.append(p[lo:hi])

    S = max(1, math.ceil(max(len(r) for r in sel_r) / 128))
    NP = S * 128

    # fm per pair: two HWC fp16 images + pad
    fmp = []
    for a, b in pairs:
        arr = np.zeros((PIX_ROWS, C), np.float16)
        arr[0:IMG_PIX] = (
            feat_map[a].transpose(1, 2, 0).reshape(IMG_PIX, C).astype(np.float16)
        )
        arr[IMG_PIX : 2 * IMG_PIX] = (
            feat_map[b].transpose(1, 2, 0).reshape(IMG_PIX, C).astype(np.float16)
        )
        fmp.append(arr)

    # device idx layout: point n -> (partition n%16 [replicated x8], col
    # 8*SG*(n//(SG*128)) + (n mod SG*128)//16)
    nn = np.arange(NP)
    g = nn // (SG * 128)
    nl = nn - g * (SG * 128)
    col_of = 8 * SG * g + nl // 16
    row_of = nl % 16

    in_maps = []
    for c in range(NCORES):
        m = len(sel_r[c])
        pixp = np.zeros(NP, np.int16)
        pixp[:m] = sel_pix[c].astype(np.int16)
        idx16 = np.zeros((16, 8 * S), np.int16)
        idx16[row_of, col_of] = pixp
        idx128 = np.tile(idx16, (8, 1))

        def dev_layout(warr):
            full = np.zeros(NP, np.float32)
            full[:m] = warr[sel_r[c], sel_j[c]]
            return np.ascontiguousarray(full.reshape(S, 128).T)

        w1d = dev_layout(w1)
        wcat = np.concatenate(
            [dev_layout(w2), dev_layout(w3), dev_layout(w4)], axis=1
        ).astype(np.float16)
        in_maps.append(
            {
                "fm": fmp[c // 2],
                "idx": idx128,
                "w1": w1d,
                "w": np.ascontiguousarray(wcat),
            }
        )
    return (sel_r, sel_j), S, in_maps


def _host_unshard(results, info, S, num_point, n):
    sel_r, sel_j = info
    out_full = np.zeros((n, num_point, C), np.float32)
    for c in range(NCORES):
        m = len(sel_r[c])
        if not m:
            continue
        o = (
            results[c]["out"]
            .astype(np.float32)
            .reshape(128, S, C)
            .transpose(1, 0, 2)
            .reshape(S * 128, C)
        )
        out_full[sel_r[c], sel_j[c]] = o[:m]
    return out_full


def kernel(feat_map, rois, offset, stride, num_point, _collect=None):
    from concourse.bass_utils import run_bass_kernel_spmd

    feat_map = np.ascontiguousarray(np.asarray(feat_map, np.float32))
    rois = np.asarray(rois, np.float32)
    offset = np.asarray(offset, np.float32)
    stride = int(stride)
    num_point = int(num_point)
    assert feat_map.shape == (B, C, H, W), feat_map.shape
    assert stride == STRIDE and num_point == NUM_POINT

    info, S, in_maps = _host_prep(feat_map, rois, offset, num_point)
    nc = _get_program(S)
    res = run_bass_kernel_spmd(nc, in_maps, core_ids=list(range(NCORES)),
                               **(_collect.pop("spmd_kwargs", {}) if _collect else {}))
    if _collect is not None:
        _collect["res"] = res
    return _host_unshard(res.results, info, S, num_point, rois.shape[0])


# revision 4
# speedup vs baseline: 6.7236x; 1.6873x over previous
"""Deformable-ROI bilinear feature gather (KeypPointBboxNet) on 8 TRN2 cores.

Strategy: fp16 feature map in HWC layout, two images packed per core (each
image replicated on 2 cores) so an int16 pixel index (max 32767 = 2*16384-1)
addresses the whole pair; the (roi,point) list of each image pair is split
point-wise between its two cores for near-perfect load balance. The host
precomputes, per point, the int16 gather index (h_low*W + w_low + img_off)
and fp16/fp32 bilinear weights, laid out exactly as the device needs them.
On device, per 640-point group:
  - one dma_gather (SWDGE) pulls the top pixel-pair (h, w..w+1) = 1KB,
  - a second dma_gather with the same indices but an AP base offset of +W
    pixels pulls the bottom pixel-pair (no separate bottom index needed),
  - the bilinear combine runs on ACT + DVE with all-fp16 tensor operands
    (single-SBUF-port 2x DVE mode, so SWDGE descriptor generation is never
    port-starved). Slots alternate between two formulations to balance the
    two engines: even in-group slots compute w1*v1 + w2*v2 + w3*v3 + w4*v4
    (1 ACT + 3 fused scalar_tensor_tensor), odd slots compute the separable
    form s = ch*top + lh*bot; out = cw*s_L + lw*s_R (2 ACT + 2 DVE).
  - the group's result is stored to HBM immediately (overlaps later groups).
Trailing pad points carry negative indices (descriptor-skipped) and zero
weights; the host inverse-routes only real points back to f32 full shape.
"""

import math

import numpy as np

B, C, H, W = 8, 256, 128, 128
N_ROIS, NUM_POINT, STRIDE = 2048, 9, 8
NCORES = 8
SG = 5  # slots (of 128 points) per dma_gather call
IMG_PIX = H * W
# two images per core + 129 pad rows (bottom gather at the last row of image 1
# reads pixels up to 2*IMG_PIX + 128 + 1).
PIX_ROWS = 2 * IMG_PIX + 2 * W + 1

_PROGRAM_CACHE: dict[int, object] = {}


def _build_program(S: int, iters: int = 1):
    import concourse.bacc as bacc
    import concourse.mybir as mybir
    import concourse.tile as tile
    from concourse.bass_types import AP

    f32 = mybir.dt.float32
    f16 = mybir.dt.float16
    i16 = mybir.dt.int16
    op = mybir.AluOpType
    Act = mybir.ActivationFunctionType

    nc = bacc.Bacc("TRN2", target_bir_lowering=False, debug=False, num_devices=NCORES)
    fm_t = nc.dram_tensor("fm", [PIX_ROWS, C], f16, kind="ExternalInput")
    idx_t = nc.dram_tensor("idx", [128, 8 * S], i16, kind="ExternalInput")
    wa_t = nc.dram_tensor("wa", [128, S], f32, kind="ExternalInput")
    wb_t = nc.dram_tensor("wb", [128, S], f32, kind="ExternalInput")
    w_t = nc.dram_tensor("w", [128, 3 * S], f16, kind="ExternalInput")
    out_t = nc.dram_tensor("out", [128, S * C], f16, kind="ExternalOutput")

    # fm viewed as overlapping [pixel, 2*C] rows with stride C (one gathered
    # element covers pixels (h,w) and (h,w+1)); the bottom view starts W
    # pixels later so the same index reads row h+1.
    top_ap = AP(fm_t, 0, [[C, 2 * IMG_PIX], [1, 2 * C]])
    bot_ap = AP(fm_t, W * C, [[C, 2 * IMG_PIX], [1, 2 * C]])

    groups = []
    s0 = 0
    while s0 < S:
        groups.append((s0, min(SG, S - s0)))
        s0 += SG

    with tile.TileContext(nc) as tc:
        with (
            tc.tile_pool(name="const", bufs=1) as cpool,
            tc.tile_pool(name="gath", bufs=2) as gpool,
            tc.tile_pool(name="work", bufs=4) as wpool,
        ):
            idx = cpool.tile([128, 8 * S], i16)
            nc.sync.dma_start(idx[:], idx_t[:])
            wa = cpool.tile([128, S], f32)
            nc.sync.dma_start(wa[:], wa_t[:])
            wb = cpool.tile([128, S], f32)
            nc.sync.dma_start(wb[:], wb_t[:])
            wt = cpool.tile([128, 3 * S], f16)
            nc.sync.dma_start(wt[:], w_t[:])

            for _it in range(iters):
                for g0, gs in groups:
                    tt = gpool.tile([128, SG, 2 * C], f16, tag="tt")
                    bt = gpool.tile([128, SG, 2 * C], f16, tag="bt")
                    ot = gpool.tile([128, SG, C], f16, tag="ot")
                    nidx = gs * 128
                    isl = idx[:, 8 * g0 : 8 * g0 + 8 * gs]
                    nc.gpsimd.dma_gather(
                        tt[:, 0:gs, :], top_ap, isl, nidx, nidx, 2 * C, elem_step=C
                    )
                    nc.gpsimd.dma_gather(
                        bt[:, 0:gs, :], bot_ap, isl, nidx, nidx, 2 * C, elem_step=C
                    )
                    for sl in range(gs):
                        s = g0 + sl
                        if sl % 2 == 0:
                            # F1: out = w1*v1 + w2*v2 + w3*v3 + w4*v4
                            # wa = w1; wt blocks = (w2, w3, w4)
                            t = wpool.tile([128, C], f16, tag="t0")
                            nc.scalar.activation(
                                t[:], tt[:, sl, 0:C], Act.Copy,
                                bias=0.0, scale=wa[:, s : s + 1],
                            )
                            a = wpool.tile([128, C], f16, tag="a0")
                            nc.vector.scalar_tensor_tensor(
                                a[:], tt[:, sl, C : 2 * C], wt[:, s : s + 1],
                                t[:], op.mult, op.add,
                            )
                            b = wpool.tile([128, C], f16, tag="b0")
                            nc.vector.scalar_tensor_tensor(
                                b[:], bt[:, sl, 0:C], wt[:, S + s : S + s + 1],
                                a[:], op.mult, op.add,
                            )
                            nc.vector.scalar_tensor_tensor(
                                ot[:, sl, :], bt[:, sl, C : 2 * C],
                                wt[:, 2 * S + s : 2 * S + s + 1], b[:],
                                op.mult, op.add,
                            )
                        else:
                            # F2: s = ch*top + lh*bot; out = cw*s_L + lw*s_R
                            # wa = ch (ACT scale), wb = lw (ACT scale),
                            # wt blocks = (lh, cw, unused)
                            t5 = wpool.tile([128, 2 * C], f16, tag="t5")
                            nc.scalar.activation(
                                t5[:], tt[:, sl, :], Act.Copy,
                                bias=0.0, scale=wa[:, s : s + 1],
                            )
                            s5 = wpool.tile([128, 2 * C], f16, tag="s5")
                            nc.vector.scalar_tensor_tensor(
                                s5[:], bt[:, sl, :], wt[:, s : s + 1],
                                t5[:], op.mult, op.add,
                            )
                            u5 = wpool.tile([128, C], f16, tag="u5")
                            nc.scalar.activation(
                                u5[:], s5[:, C : 2 * C], Act.Copy,
                                bias=0.0, scale=wb[:, s : s + 1],
                            )
                            nc.vector.scalar_tensor_tensor(
                                ot[:, sl, :], s5[:, 0:C],
                                wt[:, S + s : S + s + 1], u5[:],
                                op.mult, op.add,
                            )
                    nc.sync.dma_start(
                        out_t[:, g0 * C : (g0 + gs) * C], ot[:, 0:gs, :]
                    )

    nc.compile()
    return nc


def _get_program(S: int):
    if S not in _PROGRAM_CACHE:
        _PROGRAM_CACHE[S] = _build_program(S)
    return _PROGRAM_CACHE[S]


def _host_prep(feat_map, rois, offset, num_point):
    """Pair images, split each pair's points across its 2 cores, and build
    per-core fm / idx / weight tensors in the exact device layouts."""
    n = rois.shape[0]
    bidx = rois[:, 0].astype(np.int32)
    cnt = np.bincount(bidx, minlength=B)
    order = np.argsort(-cnt, kind="stable")
    pairs = [(int(order[k]), int(order[B - 1 - k])) for k in range(B // 2)]

    # per-point bilinear coords/weights for ALL rois (f32, matches reference)
    x1 = rois[:, 1]
    y1 = rois[:, 2]
    x2 = rois[:, 3]
    y2 = rois[:, 4]
    cx = (x1 + x2) / 2
    cy = (y1 + y2) / 2
    wx = x2 - x1 + 1
    wy = y2 - y1 + 1
    off = offset.reshape(n, num_point, 2)
    ix = (cx[:, None] + off[:, :, 0] * wx[:, None] * np.float32(0.1)) / np.float32(
        STRIDE
    )
    iy = (cy[:, None] + off[:, :, 1] * wy[:, None] * np.float32(0.1)) / np.float32(
        STRIDE
    )
    wl = np.clip(np.floor(ix), 0.0, W - 1).astype(np.float32)
    hl = np.clip(np.floor(iy), 0.0, H - 1).astype(np.float32)
    lw = np.where(wl >= W - 1, np.float32(0.0), (ix - wl).astype(np.float32))
    lh = np.where(hl >= H - 1, np.float32(0.0), (iy - hl).astype(np.float32))
    ch = 1 - lh
    cw = 1 - lw
    pix = (hl * W + wl).astype(np.int32)  # [n, P] in [0, IMG_PIX)

    # point-level assignment: pair k -> cores 2k, 2k+1
    sel_r, sel_j, sel_pix = [], [], []
    for a, b in pairs:
        ra = np.nonzero(bidx == a)[0]
        rb = np.nonzero(bidx == b)[0]
        rr = np.concatenate([ra, rb])
        im = np.concatenate(
            [np.zeros(len(ra), np.int32), np.ones(len(rb), np.int32)]
        )
        roi_rep = np.repeat(rr, num_point)
        img_rep = np.repeat(im, num_point)
        pt_j = np.tile(np.arange(num_point), len(rr))
        p = pix[roi_rep, pt_j] + img_rep * IMG_PIX
        half = (len(roi_rep) + 1) // 2
        for lo, hi in ((0, half), (half, len(roi_rep))):
            sel_r.append(roi_rep[lo:hi])
            sel_j.append(pt_j[lo:hi])
            sel_pix.append(p[lo:hi])

    S = max(1, math.ceil(max(len(r) for r in sel_r) / 128))
    NP = S * 128

    # fm per pair: two HWC fp16 images + pad
    fmp = []
    for a, b in pairs:
        arr = np.zeros((PIX_ROWS, C), np.float16)
        arr[0:IMG_PIX] = (
            feat_map[a].transpose(1, 2, 0).reshape(IMG_PIX, C).astype(np.float16)
        )
        arr[IMG_PIX : 2 * IMG_PIX] = (
            feat_map[b].transpose(1, 2, 0).reshape(IMG_PIX, C).astype(np.float16)
        )
        fmp.append(arr)

    # device idx layout: point n -> (partition n%16 [replicated x8], col
    # 8*SG*(n//(SG*128)) + (n mod SG*128)//16)
    nn = np.arange(NP)
    g = nn // (SG * 128)
    nl = nn - g * (SG * 128)
    col_of = 8 * SG * g + nl // 16
    row_of = nl % 16
    # formulation per slot: F1 if in-group slot position is even
    slot_of = nn // 128
    f1_slot = (slot_of % SG) % 2 == 0

    in_maps = []
    for c in range(NCORES):
        m = len(sel_r[c])
        pixp = np.zeros(NP, np.int16)  # pads gather pixel 0, weight 0
        pixp[:m] = sel_pix[c].astype(np.int16)
        idx16 = np.zeros((16, 8 * S), np.int16)
        idx16[row_of, col_of] = pixp
        idx128 = np.tile(idx16, (8, 1))

        def pt_vals(warr):
            full = np.zeros(NP, np.float32)
            full[:m] = warr[sel_r[c], sel_j[c]]
            return full

        p_lh = pt_vals(lh)
        p_lw = pt_vals(lw)
        p_ch = pt_vals(ch)
        p_cw = pt_vals(cw)
        # zero out pad points' weights (pt_vals already zero-pads)
        wa_pt = np.where(f1_slot, p_ch * p_cw, p_ch)  # w1 | ch
        wb_pt = np.where(f1_slot, 0.0, p_lw)  # -  | lw
        wt0 = np.where(f1_slot, p_ch * p_lw, p_lh)  # w2 | lh
        wt1 = np.where(f1_slot, p_lh * p_cw, p_cw)  # w3 | cw
        wt2 = np.where(f1_slot, p_lh * p_lw, 0.0)  # w4 | -
        # F2 pads: ch of a pad is 1-0=1 -> must zero it so output stays clean
        pad = np.arange(NP) >= m
        for arr in (wa_pt, wb_pt, wt0, wt1, wt2):
            arr[pad] = 0.0

        def dev(arrf, dt):
            return np.ascontiguousarray(arrf.reshape(S, 128).T.astype(dt))

        wcat = np.concatenate(
            [dev(wt0, np.float16), dev(wt1, np.float16), dev(wt2, np.float16)],
            axis=1,
        )
        in_maps.append(
            {
                "fm": fmp[c // 2],
                "idx": idx128,
                "wa": dev(wa_pt, np.float32),
                "wb": dev(wb_pt, np.float32),
                "w": np.ascontiguousarray(wcat),
            }
        )
    return (sel_r, sel_j), S, in_maps


def _host_unshard(results, info, S, num_point, n):
    sel_r, sel_j = info
    out_full = np.zeros((n, num_point, C), np.float32)
    for c in range(NCORES):
        m = len(sel_r[c])
        if not m:
            continue
        o = (
            results[c]["out"]
            .astype(np.float32)
            .reshape(128, S, C)
            .transpose(1, 0, 2)
            .reshape(S * 128, C)
        )
        out_full[sel_r[c], sel_j[c]] = o[:m]
    return out_full


def kernel(feat_map, rois, offset, stride, num_point, _collect=None):
    from concourse.bass_utils import run_bass_kernel_spmd

    feat_map = np.ascontiguousarray(np.asarray(feat_map, np.float32))
    rois = np.asarray(rois, np.float32)
    offset = np.asarray(offset, np.float32)
    stride = int(stride)
    num_point = int(num_point)
    assert feat_map.shape == (B, C, H, W), feat_map.shape
    assert stride == STRIDE and num_point == NUM_POINT

    info, S, in_maps = _host_prep(feat_map, rois, offset, num_point)
    nc = _get_program(S)
    res = run_bass_kernel_spmd(nc, in_maps, core_ids=list(range(NCORES)),
                               **(_collect.pop("spmd_kwargs", {}) if _collect else {}))
    if _collect is not None:
        _collect["res"] = res
    return _host_unshard(res.results, info, S, num_point, rois.shape[0])
